# revision 13
# baseline (speedup 1.0000x reference)
"""Conv2d(128->256, 3x3, VALID) + InstanceNorm2d(affine=False) + /2 on Trainium2.

Contract: kernel(**inputs) takes FULL inputs (x:[16,128,128,128] f32,
weight:[256,128,3,3] f32, bias:[256] f32) and returns the FULL output
[16,256,126,126] f32.

Strategy:
- Data-parallel over batch N=16 across 8 NeuronCores (2 images/core).
- Conv lowered to 9 accumulated matmuls per 3-row output tile: contract
  dim is C_in=128 (exactly the PE partition dim), stationary operand is
  the 128x128 weight slice for one (kh,kw,co-chunk), moving operand a
  [128, 3, 126] AP into the resident x band.
- x and weights are cast to bf16 on the host: bf16 matmuls stream at
  1 cycle/row like fp32r, but LDWEIGHTS gets Fast Weight Load (2 elems
  per cycle, disabled for fp32 dtypes), so the per-MM weight reload is
  fully hidden and the cadence drops to the ~160 ns streaming floor.
  bf16 also halves the input DMA footprint. Accumulation stays fp32 in
  PSUM; rel err ~1.5e-3, well under the 2e-2 gate.
- Tap-inner ordering: each PSUM tile finishes 9 MMs after the previous,
  so evacuation streams alongside the PE.
- Bias is skipped: InstanceNorm with affine=False makes a per-channel
  additive constant cancel exactly (shifts mean only).
- Evacuation split across engines: ACT does a single Copy with
  accum_out (sum of y); DVE computes sum(y^2) with one fused
  tensor_tensor_reduce from the evacuated SBUF tile. This keeps ACT at
  ~0.7 us/tile vs the PE's ~1.45 us/tile so PSUM banks recycle fast.
- Band-0 DMA is split 5/18/22 rows so the first row-group's input lands
  in <1 us and matmuls start almost immediately (subtile deps).
- Normalization is a single in-place DVE tensor_scalar per tile; for
  the final chunk (pure tail) it alternates ACT/DVE to halve latency.
"""

import numpy as np

import concourse.bass as bass
import concourse.tile as tile
from concourse import mybir
from concourse.vector_clock import ScopedClock

N, C_IN, H, W = 16, 128, 128, 128
C_OUT, KH, KW = 256, 3, 3
HO, WO = 126, 126
N_CORES = 8
N_PER_CORE = N // N_CORES  # 2
RG = 3                     # output rows per row-group (matmul free dim = 3*126 = 378)
NCOL = RG * WO             # 378 fp32 <= 512 (one PSUM bank)
N_RG = HO // RG            # 42 row-groups per (image, chunk)
BG = 7                     # row-groups per PSUM bank-group (7 of 8 banks in flight)
N_BG = N_RG // BG          # 6
PIX = HO * WO              # 15876
EPS = 1e-5

F32 = mybir.dt.float32
BF16 = mybir.dt.bfloat16
NP_BF16 = mybir.dt.np(BF16)


class _SplitDrainTileContext(tile.TileContext):
    """TileContext that rewrites semaphore waits to fit this walrus build,
    which caps sync-waits per instruction very low (a matmul with 2 waits
    and a drain with 3 fail codegen). Excess waits are hoisted onto
    standalone same-engine InstEventSemaphore waits placed immediately
    before the owning instruction — semantically identical (the engine
    would stall at that point anyway)."""

    def _hoist_excess_waits(self):
        nc = self.nc
        assert self.sems is not None
        id_to_handle = {h.num: h for h in self.sems.allocated().values()}
        for bb in nc.main_func.blocks:
            orig = list(bb.instructions)
            if not any(
                getattr(ins, "sync_info", None) is not None
                and len(ins.sync_info.on_wait)
                > (0 if type(ins).__name__ == "InstMatmult" else 1)
                for ins in orig
            ):
                continue
            stolen_names = set()
            new_list = []
            for ins in orig:
                si = getattr(ins, "sync_info", None)
                waits = list(si.on_wait) if si is not None and si.on_wait else []
                keep_n = 0 if type(ins).__name__ == "InstMatmult" else 1
                if len(waits) > keep_n:
                    kept = []
                    emitted = []
                    for w in waits:
                        h = id_to_handle.get(w.id)
                        if (
                            h is None
                            or w.wait_mode != "sem-ge-imm"
                            or w.wait_reg is not None
                        ):
                            kept.append(w)
                        else:
                            emitted.append((h, w))
                    while emitted and len(kept) < keep_n:
                        kept.append(emitted.pop()[1])
                    si.on_wait = kept
                    for h, w in emitted:
                        # appends to the current bb; relocated via new_list
                        wi = nc.engines[ins.engine].wait_ge(h, w.wait_value)
                        stolen_names.add(wi.ins.name)
                        new_list.append(wi.ins)
                new_list.append(ins)
            # remove the side-effect-appended copies everywhere, then install
            # the rebuilt order for this block
            for bb2 in nc.main_func.blocks:
                if bb2.name == bb.name:
                    continue
                lst = list(bb2.instructions)
                filtered = [i for i in lst if i.name not in stolen_names]
                if len(filtered) != len(lst):
                    bb2.instructions = filtered
            bb.instructions = new_list

    def _drain_and_barrier(self, tick_clock, wait_clock):
        nc = self.nc
        self._hoist_excess_waits()
        probe = nc.sync.nop()
        wait_clock.add_sem_waits(
            probe.ins, ScopedClock({None: tick_clock.global_clock})
        )
        waits = list(probe.ins.sync_info.on_wait)
        probe.ins.sync_info.on_wait = []
        assert self.sems is not None
        id_to_handle = {h.num: h for h in self.sems.allocated().values()}
        for w in waits:
            h = id_to_handle.get(w.id)
            if h is None:
                probe.ins.sync_info.on_wait.append(w)
                continue
            nc.sync.wait_ge(h, w.wait_value)
        popped = nc._tile_sem_poison_stack.pop()
        assert popped is self._sem_poison
        # Minimal teardown: the sync-engine waits above already cover every
        # completion semaphore (including the final output DMAs), so the
        # program is quiescent when sync retires. The stock drain + two
        # all-engine barriers + semaphore clear cost ~8 us of pure tail and
        # are redundant across executions: the Bass constructor preamble
        # emits dma_reset+sem_clear over the whole kernel sem range at the
        # START of every execution. Sem IDs still return to the pool.
        nc._state.prepend_free_semaphores(
            [h.num for h in self.sems.allocated().values()]
        )


def _build_nc(reps=1):
    nc = bass.Bass()
    x_d = nc.declare_dram_parameter(
        "x", [N_PER_CORE, C_IN, H, W], BF16, isOutput=False
    )
    w_d = nc.declare_dram_parameter("w", [C_IN, KH, KW, C_OUT], BF16, isOutput=False)
    # Output ships as bf16 (host casts back to f32): halves the output DMA,
    # and in particular the final chunk's post-normalize store, which is
    # pure tail (it cannot start before the instance-norm stats are in).
    o_d = nc.declare_dram_parameter(
        "out", [N_PER_CORE, C_OUT, HO, WO], BF16, isOutput=True
    )

    Copy = mybir.ActivationFunctionType.Copy
    Sqrt = mybir.ActivationFunctionType.Sqrt
    Identity = mybir.ActivationFunctionType.Identity
    mult = mybir.AluOpType.mult
    add = mybir.AluOpType.add
    subtract = mybir.AluOpType.subtract

    with _SplitDrainTileContext(nc) as tc:
        with (
            tc.tile_pool(name="xp", bufs=4) as xp,
            tc.tile_pool(name="wp", bufs=1) as wp,
            tc.tile_pool(name="yp", bufs=BG + 1) as yp,
            tc.tile_pool(name="pp", bufs=8, space="PSUM") as pp,
            tc.tile_pool(name="sqp", bufs=2) as sqp,
            tc.tile_pool(name="accp", bufs=4) as accp,
            tc.tile_pool(name="stp", bufs=22) as stp,
        ):
            wt = wp.tile([C_IN, KH, KW, C_OUT], BF16)
            nc.sync.dma_start(wt[:, :, :, 0:128], w_d[:, :, :, 0:128])
            nc.sync.dma_start(wt[:, :, :, 128:256], w_d[:, :, :, 128:256])
            epsb = wp.tile([128, 1], F32, tag="eps")
            nc.vector.memset(epsb[:], 4.0 * EPS)
            # PE warm-up: dummy matmuls on a zeroed tile keep the PE busy
            # through the HAM activity window (~3.4us) while the first input
            # DMAs land, so real matmuls start at full clock.
            zt = wp.tile([128, 128], BF16, tag="warm_src")
            nc.vector.memset(zt[:], 0.0)
            warm_ps = pp.tile([128, NCOL], F32, tag="ps", name="warm_ps")
            for _ in range(20):
                nc.tensor.matmul(warm_ps[:, 0:128], zt[:], zt[:])

            for rep in range(reps):
              for n in range(N_PER_CORE):
                  # three row-bands (rows 42b .. 42b+44), double-buffered so
                  # the next image's bands prefetch during this image's tail.
                  # Band 0 is split 5/18/22 so the first row-group's rows
                  # land fast and matmuls start ~1 us in (subtile deps).
                  bands = []
                  for b in range(3):
                      bt = xp.tile([C_IN, 45, W], BF16, tag="x", name=f"xb{b}")
                      rows = 45 if b < 2 else 44
                      if b == 0:
                          nc.sync.dma_start(bt[:, 0:5, :], x_d[n, :, 0:5, :])
                          nc.sync.dma_start(bt[:, 5:23, :], x_d[n, :, 5:23, :])
                      else:
                          nc.sync.dma_start(
                              bt[:, 0:23, :], x_d[n, :, 42 * b : 42 * b + 23, :]
                          )
                      nc.sync.dma_start(
                          bt[:, 23:rows, :], x_d[n, :, 42 * b + 23 : 42 * b + rows, :]
                      )
                      bands.append(bt)

                  for c in range(2):
                      last_chunk = rep == reps - 1 and n == N_PER_CORE - 1 and c == 1
                      sums = accp.tile([128, N_RG], F32, tag="acc")
                      sqs = accp.tile([128, N_RG], F32, tag="acc")
                      yblocks = []
                      for bg in range(N_BG):
                          yb = yp.tile([128, BG, NCOL], BF16, tag="y")
                          # tap-inner ordering: each PSUM tile finishes 9 MMs
                          # after the previous, so evacuation streams
                          # alongside the PE (bf16 FWL hides the per-MM
                          # weight reload behind the streaming matmul)
                          for j in range(BG):
                              g = bg * BG + j
                              band = bands[bg // 2]
                              ps = pp.tile([128, NCOL], F32, tag="ps", name=f"ps{bg}_{j}")
                              for t in range(KH * KW):
                                  kh, kw = divmod(t, KW)
                                  lhs = wt[:, kh, kw, c * 128 : (c + 1) * 128]
                                  r0 = RG * g + kh - 42 * (bg // 2)
                                  rhs = band[:, r0 : r0 + RG, kw : kw + WO]
                                  nc.tensor.matmul(
                                      ps[:],
                                      lhs,
                                      rhs,
                                      start=(t == 0),
                                      stop=(t == KH * KW - 1),
                                  )
                              # ACT evacuates + accumulates sum(y); DVE
                              # computes sum(y^2) in one fused pass off the
                              # SBUF copy, freeing the PSUM bank after a
                              # single ACT read.
                              nc.scalar.activation(
                                  yb[:, j, :],
                                  ps[:],
                                  Copy,
                                  accum_out=sums[:, g : g + 1],
                              )
                              sq = sqp.tile([128, NCOL], BF16, tag="sq", name=f"sq{j}")
                              nc.vector.scalar_tensor_tensor(
                                  sq[:],
                                  yb[:, j, :],
                                  1.0,
                                  yb[:, j, :],
                                  op0=mult,
                                  op1=mult,
                                  accum_out=sqs[:, g : g + 1],
                              )
                          yblocks.append(yb)

                      # Per-(n, channel) stats over all 15876 pixels.
                      # Chain kept short: it is on the critical path for the
                      # final chunk's (pure-tail) normalize + store.
                      s1 = stp.tile([128, 1], F32, tag="st")
                      nc.vector.tensor_reduce(
                          s1[:], sums[:], axis=mybir.AxisListType.X, op=add
                      )
                      s2 = stp.tile([128, 1], F32, tag="st")
                      nc.vector.tensor_reduce(
                          s2[:], sqs[:], axis=mybir.AxisListType.X, op=add
                      )
                      mean = stp.tile([128, 1], F32, tag="st")
                      nc.vector.tensor_scalar_mul(mean[:], s1[:], 1.0 / PIX)
                      msq = stp.tile([128, 1], F32, tag="st")
                      nc.vector.tensor_mul(msq[:], mean[:], mean[:])
                      var = stp.tile([128, 1], F32, tag="st")
                      nc.vector.scalar_tensor_tensor(
                          var[:], s2[:], 1.0 / PIX, msq[:],
                          op0=mult, op1=subtract,
                      )
                      # alpha = rsqrt(var+eps)/2 = 1/sqrt(4*var + 4*eps)
                      std2 = stp.tile([128, 1], F32, tag="st")
                      nc.scalar.activation(
                          std2[:], var[:], Sqrt, bias=epsb[:], scale=4.0
                      )
                      alpha = stp.tile([128, 1], F32, tag="st")
                      nc.vector.reciprocal(alpha[:], std2[:])
                      nmalpha = stp.tile([128, 1], F32, tag="st")
                      nc.vector.scalar_tensor_tensor(
                          nmalpha[:], mean[:], -1.0, alpha[:],
                          op0=mult, op1=mult,
                      )

                      for bg, yb in enumerate(yblocks):
                          for j in range(BG):
                              if last_chunk and j % 3 == 2:
                                  nc.scalar.activation(
                                      yb[:, j, :],
                                      yb[:, j, :],
                                      Identity,
                                      bias=nmalpha[:],
                                      scale=alpha[:],
                                  )
                              else:
                                  nc.vector.tensor_scalar(
                                      yb[:, j, :],
                                      yb[:, j, :],
                                      alpha[:],
                                      nmalpha[:],
                                      op0=mult,
                                      op1=add,
                                  )
                              if last_chunk and j == 3:
                                  # early half-group store: the tail DMA is
                                  # bandwidth-bound, start it sooner
                                  nc.sync.dma_start(
                                      o_d[
                                          n,
                                          c * 128 : (c + 1) * 128,
                                          bg * BG * RG : bg * BG * RG + 4 * RG,
                                          :,
                                      ],
                                      yb[:, 0:4, :],
                                  )
                          if last_chunk:
                              nc.sync.dma_start(
                                  o_d[
                                      n,
                                      c * 128 : (c + 1) * 128,
                                      bg * BG * RG + 4 * RG : (bg + 1) * BG * RG,
                                      :,
                                  ],
                                  yb[:, 4:BG, :],
                              )
                          else:
                              nc.sync.dma_start(
                                  o_d[
                                      n,
                                      c * 128 : (c + 1) * 128,
                                      bg * BG * RG : (bg + 1) * BG * RG,
                                      :,
                                  ],
                                  yb[:],
                              )
    return nc


def _per_core_inputs(x, weight):
    """Shard + dtype-convert FULL inputs into the per-core in_map lists."""
    x16 = np.ascontiguousarray(np.asarray(x)).astype(NP_BF16)
    wt = np.ascontiguousarray(
        np.asarray(weight, dtype=np.float32).transpose(1, 2, 3, 0)
    ).astype(NP_BF16)
    return {
        "x": [x16[c * N_PER_CORE : (c + 1) * N_PER_CORE] for c in range(N_CORES)],
        "w": [wt] * N_CORES,
    }


_CACHED = None


def _get_exec(reps=1):
    """Build the Bass program once and wrap it in a persistent jitted
    shard_map executor (mirrors bass2jax.run_bass_via_pjrt, but without
    donation so the callable can be re-invoked for timing)."""
    global _CACHED
    if _CACHED is not None and _CACHED[5] == reps:
        return _CACHED

    import jax
    from jax.experimental.shard_map import shard_map
    from jax.sharding import Mesh, PartitionSpec

    from concourse import bass2jax

    bass2jax.install_neuronx_cc_hook()
    nc = _build_nc(reps)

    partition_name = (
        nc.partition_id_tensor.name if nc.partition_id_tensor else None
    )
    in_names = []
    out_names = []
    out_avals = []
    for alloc in nc.m.functions[0].allocations:
        if not isinstance(alloc, mybir.MemoryLocationSet):
            continue
        name = alloc.memorylocations[0].name
        if alloc.kind == "ExternalInput":
            if name != partition_name:
                in_names.append(name)
        elif alloc.kind == "ExternalOutput":
            out_names.append(name)
            out_avals.append(
                jax.core.ShapedArray(
                    tuple(alloc.tensor_shape), mybir.dt.np(alloc.dtype)
                )
            )
    n_params = len(in_names)
    all_in_names = in_names + out_names
    if partition_name is not None:
        all_in_names = all_in_names + [partition_name]

    def _body(*args):
        operands = list(args)
        if partition_name is not None:
            operands.append(bass2jax.partition_id_tensor())
        outs = bass2jax._bass_exec_p.bind(
            *operands,
            out_avals=tuple(out_avals),
            in_names=tuple(all_in_names),
            out_names=tuple(out_names),
            lowering_input_output_aliases=(),
            sim_require_finite=True,
            sim_require_nnan=True,
            nc=nc,
        )
        return tuple(outs)

    devices = jax.devices()[:N_CORES]
    mesh = Mesh(np.asarray(devices), ("core",))
    n_outs = len(out_names)
    sharded = jax.jit(
        shard_map(
            _body,
            mesh=mesh,
            in_specs=(PartitionSpec("core"),) * (n_params + n_outs),
            out_specs=(PartitionSpec("core"),) * n_outs,
            check_rep=False,
        ),
        keep_unused=True,
    )
    zeros = [
        np.zeros((N_CORES * a.shape[0], *a.shape[1:]), a.dtype) for a in out_avals
    ]
    _CACHED = (sharded, in_names, out_names, out_avals, zeros, reps)
    return _CACHED


def _run(per_core_inputs):
    """per_core_inputs: dict name -> list of 8 per-core arrays.
    Returns dict name -> list of 8 per-core outputs."""
    sharded, in_names, out_names, out_avals, zeros, _ = _get_exec()
    concat_in = [
        np.concatenate([np.asarray(per_core_inputs[nm][c]) for c in range(N_CORES)], axis=0)
        for nm in in_names
    ]
    out_arrs = sharded(*concat_in, *zeros)
    return {
        nm: np.asarray(out_arrs[i]).reshape(N_CORES, *out_avals[i].shape)
        for i, nm in enumerate(out_names)
    }


def kernel(x, weight, bias):
    # bias is mathematically a no-op under InstanceNorm(affine=False).
    del bias
    per_core = _per_core_inputs(x, weight)
    outs = _run(per_core)["out"]  # [8, 2, 256, 126, 126] bf16
    return outs.reshape(N, C_OUT, HO, WO).astype(np.float32)


# revision 15
# speedup vs baseline: 1.0216x; 1.0216x over previous
"""Conv2d(128->256, 3x3, VALID) + InstanceNorm2d(affine=False) + /2 on Trainium2.

Contract: kernel(**inputs) takes FULL inputs (x:[16,128,128,128] f32,
weight:[256,128,3,3] f32, bias:[256] f32) and returns the FULL output
[16,256,126,126] f32.

Strategy:
- Data-parallel over batch N=16 across 8 NeuronCores (2 images/core).
- Conv lowered to 9 accumulated matmuls per 3-row output tile: contract
  dim is C_in=128 (exactly the PE partition dim), stationary operand is
  the 128x128 weight slice for one (kh,kw,co-chunk), moving operand a
  [128, 3, 126] AP into the resident x band.
- x and weights are cast to bf16 on the host: bf16 matmuls stream at
  1 cycle/row like fp32r, but LDWEIGHTS gets Fast Weight Load (2 elems
  per cycle, disabled for fp32 dtypes), so the per-MM weight reload is
  fully hidden and the cadence drops to the ~160 ns streaming floor.
  bf16 also halves the input DMA footprint. Accumulation stays fp32 in
  PSUM; rel err ~1.5e-3, well under the 2e-2 gate.
- Tap-inner ordering: each PSUM tile finishes 9 MMs after the previous,
  so evacuation streams alongside the PE.
- Bias is skipped: InstanceNorm with affine=False makes a per-channel
  additive constant cancel exactly (shifts mean only).
- Evacuation split across engines: ACT does a single Copy with
  accum_out (sum of y); DVE computes sum(y^2) with one fused
  tensor_tensor_reduce from the evacuated SBUF tile. This keeps ACT at
  ~0.7 us/tile vs the PE's ~1.45 us/tile so PSUM banks recycle fast.
- Band-0 DMA is split 5/18/22 rows so the first row-group's input lands
  in <1 us and matmuls start almost immediately (subtile deps).
- Normalization is a single in-place DVE tensor_scalar per tile
  (y*alpha + nmalpha); for the final chunk (pure tail) it alternates
  ACT/DVE and issues half-bank-group output DMAs to hide the
  bandwidth-bound final store.
- Minimal teardown: the stock TileContext drain + two all-engine
  barriers + semaphore clear cost ~8 us of pure tail and are redundant
  across executions because the Bass preamble resets DMA/semaphore
  state at the start of every execution (verified over 300+
  re-invocations).
"""

import numpy as np

import concourse.bass as bass
import concourse.tile as tile
from concourse import mybir
from concourse.vector_clock import ScopedClock

N, C_IN, H, W = 16, 128, 128, 128
C_OUT, KH, KW = 256, 3, 3
HO, WO = 126, 126
N_CORES = 8
N_PER_CORE = N // N_CORES  # 2
RG = 3                     # output rows per row-group (matmul free dim = 3*126 = 378)
NCOL = RG * WO             # 378 fp32 <= 512 (one PSUM bank)
N_RG = HO // RG            # 42 row-groups per (image, chunk)
BG = 7                     # row-groups per PSUM bank-group (7 of 8 banks in flight)
N_BG = N_RG // BG          # 6
PIX = HO * WO              # 15876
EPS = 1e-5

F32 = mybir.dt.float32
BF16 = mybir.dt.bfloat16
NP_BF16 = mybir.dt.np(BF16)


class _SplitDrainTileContext(tile.TileContext):
    """TileContext that rewrites semaphore waits to fit this walrus build,
    which caps sync-waits per instruction very low (a matmul with 2 waits
    and a drain with 3 fail codegen). Excess waits are hoisted onto
    standalone same-engine InstEventSemaphore waits placed immediately
    before the owning instruction — semantically identical (the engine
    would stall at that point anyway)."""

    def _hoist_excess_waits(self):
        nc = self.nc
        assert self.sems is not None
        id_to_handle = {h.num: h for h in self.sems.allocated().values()}
        for bb in nc.main_func.blocks:
            orig = list(bb.instructions)
            if not any(
                getattr(ins, "sync_info", None) is not None
                and len(ins.sync_info.on_wait)
                > (0 if type(ins).__name__ == "InstMatmult" else 1)
                for ins in orig
            ):
                continue
            stolen_names = set()
            new_list = []
            for ins in orig:
                si = getattr(ins, "sync_info", None)
                waits = list(si.on_wait) if si is not None and si.on_wait else []
                keep_n = 0 if type(ins).__name__ == "InstMatmult" else 1
                if len(waits) > keep_n:
                    kept = []
                    emitted = []
                    for w in waits:
                        h = id_to_handle.get(w.id)
                        if (
                            h is None
                            or w.wait_mode != "sem-ge-imm"
                            or w.wait_reg is not None
                        ):
                            kept.append(w)
                        else:
                            emitted.append((h, w))
                    while emitted and len(kept) < keep_n:
                        kept.append(emitted.pop()[1])
                    si.on_wait = kept
                    for h, w in emitted:
                        # appends to the current bb; relocated via new_list
                        wi = nc.engines[ins.engine].wait_ge(h, w.wait_value)
                        stolen_names.add(wi.ins.name)
                        new_list.append(wi.ins)
                new_list.append(ins)
            # remove the side-effect-appended copies everywhere, then install
            # the rebuilt order for this block
            for bb2 in nc.main_func.blocks:
                if bb2.name == bb.name:
                    continue
                lst = list(bb2.instructions)
                filtered = [i for i in lst if i.name not in stolen_names]
                if len(filtered) != len(lst):
                    bb2.instructions = filtered
            bb.instructions = new_list

    def _drain_and_barrier(self, tick_clock, wait_clock):
        nc = self.nc
        self._hoist_excess_waits()
        probe = nc.sync.nop()
        wait_clock.add_sem_waits(
            probe.ins, ScopedClock({None: tick_clock.global_clock})
        )
        waits = list(probe.ins.sync_info.on_wait)
        probe.ins.sync_info.on_wait = []
        assert self.sems is not None
        id_to_handle = {h.num: h for h in self.sems.allocated().values()}
        for w in waits:
            h = id_to_handle.get(w.id)
            if h is None:
                probe.ins.sync_info.on_wait.append(w)
                continue
            nc.sync.wait_ge(h, w.wait_value)
        popped = nc._tile_sem_poison_stack.pop()
        assert popped is self._sem_poison
        # Minimal teardown: the sync-engine waits above already cover every
        # completion semaphore (including the final output DMAs), so the
        # program is quiescent when sync retires. The stock drain + two
        # all-engine barriers + semaphore clear cost ~8 us of pure tail and
        # are redundant across executions: the Bass constructor preamble
        # emits dma_reset+sem_clear over the whole kernel sem range at the
        # START of every execution. Sem IDs still return to the pool.
        nc._state.prepend_free_semaphores(
            [h.num for h in self.sems.allocated().values()]
        )


def _build_nc(reps=1):
    nc = bass.Bass()
    x_d = nc.declare_dram_parameter(
        "x", [N_PER_CORE, C_IN, H, W], BF16, isOutput=False
    )
    w_d = nc.declare_dram_parameter("w", [C_IN, KH, KW, C_OUT], BF16, isOutput=False)
    # Output ships as bf16 (host casts back to f32): halves the output DMA,
    # and in particular the final chunk's post-normalize store, which is
    # pure tail (it cannot start before the instance-norm stats are in).
    o_d = nc.declare_dram_parameter(
        "out", [N_PER_CORE, C_OUT, HO, WO], BF16, isOutput=True
    )

    Copy = mybir.ActivationFunctionType.Copy
    Sqrt = mybir.ActivationFunctionType.Sqrt
    Identity = mybir.ActivationFunctionType.Identity
    mult = mybir.AluOpType.mult
    add = mybir.AluOpType.add
    subtract = mybir.AluOpType.subtract

    with _SplitDrainTileContext(nc) as tc:
        with (
            tc.tile_pool(name="xp", bufs=6) as xp,
            tc.tile_pool(name="wp", bufs=1) as wp,
            tc.tile_pool(name="yp", bufs=BG + 1) as yp,
            tc.tile_pool(name="pp", bufs=8, space="PSUM") as pp,
            tc.tile_pool(name="sqp", bufs=2) as sqp,
            tc.tile_pool(name="accp", bufs=4) as accp,
            tc.tile_pool(name="stp", bufs=22) as stp,
        ):
            wt = wp.tile([C_IN, KH, KW, C_OUT], BF16)
            nc.sync.dma_start(wt[:, :, :, 0:128], w_d[:, :, :, 0:128])
            nc.sync.dma_start(wt[:, :, :, 128:256], w_d[:, :, :, 128:256])
            epsb = wp.tile([128, 1], F32, tag="eps")
            nc.vector.memset(epsb[:], 4.0 * EPS)
            # PE warm-up: dummy matmuls on a zeroed tile keep the PE busy
            # through the HAM activity window (~3.4us) while the first input
            # DMAs land, so real matmuls start at full clock.
            zt = wp.tile([128, 128], BF16, tag="warm_src")
            nc.vector.memset(zt[:], 0.0)
            warm_ps = pp.tile([128, NCOL], F32, tag="ps", name="warm_ps")
            for _ in range(20):
                nc.tensor.matmul(warm_ps[:, 0:128], zt[:], zt[:])

            for rep in range(reps):
              for n in range(N_PER_CORE):
                  # three row-bands (rows 42b .. 42b+44), double-buffered so
                  # the next image's bands prefetch during this image's tail.
                  # Band 0 is split 5/18/22 so the first row-group's rows
                  # land fast and matmuls start ~1 us in (subtile deps).
                  bands = []
                  for b in range(3):
                      bt = xp.tile([C_IN, 45, W], BF16, tag="x", name=f"xb{b}")
                      rows = 45 if b < 2 else 44
                      if b == 0:
                          nc.sync.dma_start(bt[:, 0:5, :], x_d[n, :, 0:5, :])
                          nc.sync.dma_start(bt[:, 5:23, :], x_d[n, :, 5:23, :])
                      else:
                          nc.sync.dma_start(
                              bt[:, 0:23, :], x_d[n, :, 42 * b : 42 * b + 23, :]
                          )
                      nc.sync.dma_start(
                          bt[:, 23:rows, :], x_d[n, :, 42 * b + 23 : 42 * b + rows, :]
                      )
                      bands.append(bt)

                  for c in range(2):
                      last_chunk = rep == reps - 1 and n == N_PER_CORE - 1 and c == 1
                      sums = accp.tile([128, N_RG], F32, tag="acc")
                      sqs = accp.tile([128, N_RG], F32, tag="acc")
                      yblocks = []
                      for bg in range(N_BG):
                          yb = yp.tile([128, BG, NCOL], BF16, tag="y")
                          # tap-inner ordering: each PSUM tile finishes 9 MMs
                          # after the previous, so evacuation streams
                          # alongside the PE (bf16 FWL hides the per-MM
                          # weight reload behind the streaming matmul)
                          for j in range(BG):
                              g = bg * BG + j
                              band = bands[bg // 2]
                              ps = pp.tile([128, NCOL], F32, tag="ps", name=f"ps{bg}_{j}")
                              for t in range(KH * KW):
                                  kh, kw = divmod(t, KW)
                                  lhs = wt[:, kh, kw, c * 128 : (c + 1) * 128]
                                  r0 = RG * g + kh - 42 * (bg // 2)
                                  rhs = band[:, r0 : r0 + RG, kw : kw + WO]
                                  nc.tensor.matmul(
                                      ps[:],
                                      lhs,
                                      rhs,
                                      start=(t == 0),
                                      stop=(t == KH * KW - 1),
                                  )
                              # ACT evacuates + accumulates sum(y); DVE
                              # computes sum(y^2) in one fused pass off the
                              # SBUF copy, freeing the PSUM bank after a
                              # single ACT read.
                              nc.scalar.activation(
                                  yb[:, j, :],
                                  ps[:],
                                  Copy,
                                  accum_out=sums[:, g : g + 1],
                              )
                              sq = sqp.tile([128, NCOL], BF16, tag="sq", name=f"sq{j}")
                              nc.vector.scalar_tensor_tensor(
                                  sq[:],
                                  yb[:, j, :],
                                  1.0,
                                  yb[:, j, :],
                                  op0=mult,
                                  op1=mult,
                                  accum_out=sqs[:, g : g + 1],
                              )
                          yblocks.append(yb)

                      # Per-(n, channel) stats over all 15876 pixels.
                      # Chain kept short: it is on the critical path for the
                      # final chunk's (pure-tail) normalize + store.
                      s1 = stp.tile([128, 1], F32, tag="st")
                      nc.vector.tensor_reduce(
                          s1[:], sums[:], axis=mybir.AxisListType.X, op=add
                      )
                      s2 = stp.tile([128, 1], F32, tag="st")
                      nc.vector.tensor_reduce(
                          s2[:], sqs[:], axis=mybir.AxisListType.X, op=add
                      )
                      mean = stp.tile([128, 1], F32, tag="st")
                      nc.vector.tensor_scalar_mul(mean[:], s1[:], 1.0 / PIX)
                      msq = stp.tile([128, 1], F32, tag="st")
                      nc.vector.tensor_mul(msq[:], mean[:], mean[:])
                      var = stp.tile([128, 1], F32, tag="st")
                      nc.vector.scalar_tensor_tensor(
                          var[:], s2[:], 1.0 / PIX, msq[:],
                          op0=mult, op1=subtract,
                      )
                      # alpha = rsqrt(var+eps)/2 = 1/sqrt(4*var + 4*eps)
                      std2 = stp.tile([128, 1], F32, tag="st")
                      nc.scalar.activation(
                          std2[:], var[:], Sqrt, bias=epsb[:], scale=4.0
                      )
                      alpha = stp.tile([128, 1], F32, tag="st")
                      nc.vector.reciprocal(alpha[:], std2[:])
                      nmalpha = stp.tile([128, 1], F32, tag="st")
                      nc.vector.scalar_tensor_tensor(
                          nmalpha[:], mean[:], -1.0, alpha[:],
                          op0=mult, op1=mult,
                      )

                      for bg, yb in enumerate(yblocks):
                          for j in range(BG):
                              if last_chunk and j % 3 == 2:
                                  nc.scalar.activation(
                                      yb[:, j, :],
                                      yb[:, j, :],
                                      Identity,
                                      bias=nmalpha[:],
                                      scale=alpha[:],
                                  )
                              else:
                                  nc.vector.tensor_scalar(
                                      yb[:, j, :],
                                      yb[:, j, :],
                                      alpha[:],
                                      nmalpha[:],
                                      op0=mult,
                                      op1=add,
                                  )
                              if last_chunk and j == 3:
                                  # early half-group store: the tail DMA is
                                  # bandwidth-bound, start it sooner
                                  nc.sync.dma_start(
                                      o_d[
                                          n,
                                          c * 128 : (c + 1) * 128,
                                          bg * BG * RG : bg * BG * RG + 4 * RG,
                                          :,
                                      ],
                                      yb[:, 0:4, :],
                                  )
                          if last_chunk:
                              nc.sync.dma_start(
                                  o_d[
                                      n,
                                      c * 128 : (c + 1) * 128,
                                      bg * BG * RG + 4 * RG : (bg + 1) * BG * RG,
                                      :,
                                  ],
                                  yb[:, 4:BG, :],
                              )
                          else:
                              nc.sync.dma_start(
                                  o_d[
                                      n,
                                      c * 128 : (c + 1) * 128,
                                      bg * BG * RG : (bg + 1) * BG * RG,
                                      :,
                                  ],
                                  yb[:],
                              )
    return nc


def _per_core_inputs(x, weight):
    """Shard + dtype-convert FULL inputs into the per-core in_map lists."""
    x16 = np.ascontiguousarray(np.asarray(x)).astype(NP_BF16)
    wt = np.ascontiguousarray(
        np.asarray(weight, dtype=np.float32).transpose(1, 2, 3, 0)
    ).astype(NP_BF16)
    return {
        "x": [x16[c * N_PER_CORE : (c + 1) * N_PER_CORE] for c in range(N_CORES)],
        "w": [wt] * N_CORES,
    }


_CACHED = None


def _get_exec(reps=1):
    """Build the Bass program once and wrap it in a persistent jitted
    shard_map executor (mirrors bass2jax.run_bass_via_pjrt, but without
    donation so the callable can be re-invoked for timing)."""
    global _CACHED
    if _CACHED is not None and _CACHED[5] == reps:
        return _CACHED

    import jax
    from jax.experimental.shard_map import shard_map
    from jax.sharding import Mesh, PartitionSpec

    from concourse import bass2jax

    bass2jax.install_neuronx_cc_hook()
    nc = _build_nc(reps)

    partition_name = (
        nc.partition_id_tensor.name if nc.partition_id_tensor else None
    )
    in_names = []
    out_names = []
    out_avals = []
    for alloc in nc.m.functions[0].allocations:
        if not isinstance(alloc, mybir.MemoryLocationSet):
            continue
        name = alloc.memorylocations[0].name
        if alloc.kind == "ExternalInput":
            if name != partition_name:
                in_names.append(name)
        elif alloc.kind == "ExternalOutput":
            out_names.append(name)
            out_avals.append(
                jax.core.ShapedArray(
                    tuple(alloc.tensor_shape), mybir.dt.np(alloc.dtype)
                )
            )
    n_params = len(in_names)
    all_in_names = in_names + out_names
    if partition_name is not None:
        all_in_names = all_in_names + [partition_name]

    def _body(*args):
        operands = list(args)
        if partition_name is not None:
            operands.append(bass2jax.partition_id_tensor())
        outs = bass2jax._bass_exec_p.bind(
            *operands,
            out_avals=tuple(out_avals),
            in_names=tuple(all_in_names),
            out_names=tuple(out_names),
            lowering_input_output_aliases=(),
            sim_require_finite=True,
            sim_require_nnan=True,
            nc=nc,
        )
        return tuple(outs)

    devices = jax.devices()[:N_CORES]
    mesh = Mesh(np.asarray(devices), ("core",))
    n_outs = len(out_names)
    sharded = jax.jit(
        shard_map(
            _body,
            mesh=mesh,
            in_specs=(PartitionSpec("core"),) * (n_params + n_outs),
            out_specs=(PartitionSpec("core"),) * n_outs,
            check_rep=False,
        ),
        keep_unused=True,
    )
    zeros = [
        np.zeros((N_CORES * a.shape[0], *a.shape[1:]), a.dtype) for a in out_avals
    ]
    _CACHED = (sharded, in_names, out_names, out_avals, zeros, reps)
    return _CACHED


def _run(per_core_inputs):
    """per_core_inputs: dict name -> list of 8 per-core arrays.
    Returns dict name -> list of 8 per-core outputs."""
    sharded, in_names, out_names, out_avals, zeros, _ = _get_exec()
    concat_in = [
        np.concatenate([np.asarray(per_core_inputs[nm][c]) for c in range(N_CORES)], axis=0)
        for nm in in_names
    ]
    out_arrs = sharded(*concat_in, *zeros)
    return {
        nm: np.asarray(out_arrs[i]).reshape(N_CORES, *out_avals[i].shape)
        for i, nm in enumerate(out_names)
    }


def kernel(x, weight, bias):
    # bias is mathematically a no-op under InstanceNorm(affine=False).
    del bias
    per_core = _per_core_inputs(x, weight)
    outs = _run(per_core)["out"]  # [8, 2, 256, 126, 126] bf16
    return outs.reshape(N, C_OUT, HO, WO).astype(np.float32)


# revision 18
# speedup vs baseline: 1.0655x; 1.0430x over previous
"""Conv2d(128->256, 3x3, VALID) + InstanceNorm2d(affine=False) + /2 on Trainium2.

Contract: kernel(**inputs) takes FULL inputs (x:[16,128,128,128] f32,
weight:[256,128,3,3] f32, bias:[256] f32) and returns the FULL output
[16,256,126,126] f32.

Strategy:
- Data-parallel over batch N=16 across 8 NeuronCores (2 images/core).
- Conv lowered to 9 accumulated matmuls per 3-row output tile: contract
  dim is C_in=128 (exactly the PE partition dim), stationary operand is
  the 128x128 weight slice for one (kh,kw,co-chunk), moving operand a
  [128, 3, 126] AP into the resident x band.
- x and weights are cast to bf16 on the host: bf16 matmuls stream at
  1 cycle/row like fp32r, but LDWEIGHTS gets Fast Weight Load (2 elems
  per cycle, disabled for fp32 dtypes), so the per-MM weight reload is
  fully hidden and the cadence drops to the ~160 ns streaming floor.
  bf16 also halves the input DMA footprint. Accumulation stays fp32 in
  PSUM; rel err ~1.5e-3, well under the 2e-2 gate.
- Tap-inner ordering: each PSUM tile finishes 9 MMs after the previous,
  so evacuation streams alongside the PE.
- Bias is skipped: InstanceNorm with affine=False makes a per-channel
  additive constant cancel exactly (shifts mean only).
- Evacuation split across engines: ACT does a single Copy with
  accum_out (sum of y); DVE computes sum(y^2) with one fused
  tensor_tensor_reduce from the evacuated SBUF tile. This keeps ACT at
  ~0.7 us/tile vs the PE's ~1.45 us/tile so PSUM banks recycle fast.
- Band-0 DMA is split 5/18/22 rows so the first row-group's input lands
  in <1 us and matmuls start almost immediately (subtile deps).
- Normalization is a single in-place DVE tensor_scalar per tile
  (y*alpha + nmalpha); for the final chunk (pure tail) it alternates
  ACT/DVE and issues half-bank-group output DMAs to hide the
  bandwidth-bound final store.
- Minimal teardown: the stock TileContext drain + two all-engine
  barriers + semaphore clear cost ~8 us of pure tail and are redundant
  across executions because the Bass preamble resets DMA/semaphore
  state at the start of every execution (verified over 300+
  re-invocations).
"""

import numpy as np

import concourse.bass as bass
import concourse.tile as tile
from concourse import mybir
from concourse.vector_clock import ScopedClock

N, C_IN, H, W = 16, 128, 128, 128
C_OUT, KH, KW = 256, 3, 3
HO, WO = 126, 126
N_CORES = 8
N_PER_CORE = N // N_CORES  # 2
RG = 3                     # output rows per row-group (matmul free dim = 3*126 = 378)
NCOL = RG * WO             # 378 fp32 <= 512 (one PSUM bank)
N_RG = HO // RG            # 42 row-groups per (image, chunk)
BG = 7                     # row-groups per PSUM bank-group (7 of 8 banks in flight)
N_BG = N_RG // BG          # 6
PIX = HO * WO              # 15876
EPS = 1e-5

F32 = mybir.dt.float32
BF16 = mybir.dt.bfloat16
NP_BF16 = mybir.dt.np(BF16)


class _SplitDrainTileContext(tile.TileContext):
    """TileContext that rewrites semaphore waits to fit this walrus build,
    which caps sync-waits per instruction very low (a matmul with 2 waits
    and a drain with 3 fail codegen). Excess waits are hoisted onto
    standalone same-engine InstEventSemaphore waits placed immediately
    before the owning instruction — semantically identical (the engine
    would stall at that point anyway)."""

    def _hoist_excess_waits(self):
        nc = self.nc
        assert self.sems is not None
        id_to_handle = {h.num: h for h in self.sems.allocated().values()}
        for bb in nc.main_func.blocks:
            orig = list(bb.instructions)
            if not any(
                getattr(ins, "sync_info", None) is not None
                and len(ins.sync_info.on_wait)
                > (0 if type(ins).__name__ == "InstMatmult" else 1)
                for ins in orig
            ):
                continue
            stolen_names = set()
            new_list = []
            for ins in orig:
                si = getattr(ins, "sync_info", None)
                waits = list(si.on_wait) if si is not None and si.on_wait else []
                keep_n = 0 if type(ins).__name__ == "InstMatmult" else 1
                if len(waits) > keep_n:
                    kept = []
                    emitted = []
                    for w in waits:
                        h = id_to_handle.get(w.id)
                        if (
                            h is None
                            or w.wait_mode != "sem-ge-imm"
                            or w.wait_reg is not None
                        ):
                            kept.append(w)
                        else:
                            emitted.append((h, w))
                    while emitted and len(kept) < keep_n:
                        kept.append(emitted.pop()[1])
                    si.on_wait = kept
                    for h, w in emitted:
                        # appends to the current bb; relocated via new_list
                        wi = nc.engines[ins.engine].wait_ge(h, w.wait_value)
                        stolen_names.add(wi.ins.name)
                        new_list.append(wi.ins)
                new_list.append(ins)
            # remove the side-effect-appended copies everywhere, then install
            # the rebuilt order for this block
            for bb2 in nc.main_func.blocks:
                if bb2.name == bb.name:
                    continue
                lst = list(bb2.instructions)
                filtered = [i for i in lst if i.name not in stolen_names]
                if len(filtered) != len(lst):
                    bb2.instructions = filtered
            bb.instructions = new_list

    def _drain_and_barrier(self, tick_clock, wait_clock):
        nc = self.nc
        self._hoist_excess_waits()
        probe = nc.sync.nop()
        wait_clock.add_sem_waits(
            probe.ins, ScopedClock({None: tick_clock.global_clock})
        )
        waits = list(probe.ins.sync_info.on_wait)
        probe.ins.sync_info.on_wait = []
        assert self.sems is not None
        id_to_handle = {h.num: h for h in self.sems.allocated().values()}
        for w in waits:
            h = id_to_handle.get(w.id)
            if h is None:
                probe.ins.sync_info.on_wait.append(w)
                continue
            nc.sync.wait_ge(h, w.wait_value)
        popped = nc._tile_sem_poison_stack.pop()
        assert popped is self._sem_poison
        # Minimal teardown: the sync-engine waits above already cover every
        # completion semaphore (including the final output DMAs), so the
        # program is quiescent when sync retires. The stock drain + two
        # all-engine barriers + semaphore clear cost ~8 us of pure tail and
        # are redundant across executions: the Bass constructor preamble
        # emits dma_reset+sem_clear over the whole kernel sem range at the
        # START of every execution. Sem IDs still return to the pool.
        nc._state.prepend_free_semaphores(
            [h.num for h in self.sems.allocated().values()]
        )


def _build_nc(reps=1):
    nc = bass.Bass()
    x_d = nc.declare_dram_parameter(
        "x", [N_PER_CORE, C_IN, H, W], BF16, isOutput=False
    )
    w_d = nc.declare_dram_parameter("w", [C_IN, KH, KW, C_OUT], BF16, isOutput=False)
    # Output ships as bf16 (host casts back to f32): halves the output DMA,
    # and in particular the final chunk's post-normalize store, which is
    # pure tail (it cannot start before the instance-norm stats are in).
    o_d = nc.declare_dram_parameter(
        "out", [N_PER_CORE, C_OUT, HO, WO], BF16, isOutput=True
    )

    Copy = mybir.ActivationFunctionType.Copy
    Sqrt = mybir.ActivationFunctionType.Sqrt
    Identity = mybir.ActivationFunctionType.Identity
    mult = mybir.AluOpType.mult
    add = mybir.AluOpType.add
    subtract = mybir.AluOpType.subtract

    with _SplitDrainTileContext(nc) as tc:
        with (
            tc.tile_pool(name="xp", bufs=6) as xp,
            tc.tile_pool(name="wp", bufs=1) as wp,
            tc.tile_pool(name="yp", bufs=BG + 1) as yp,
            tc.tile_pool(name="pp", bufs=8, space="PSUM") as pp,
            tc.tile_pool(name="sqp", bufs=2) as sqp,
            tc.tile_pool(name="accp", bufs=4) as accp,
            tc.tile_pool(name="stp", bufs=22) as stp,
        ):
            wt = wp.tile([C_IN, KH, KW, C_OUT], BF16)
            # Only the c0 half gates the first matmul; the c1 half is not
            # read until ~70us in, so it is issued after image 0's band-0
            # rows (below) to get the first real matmul started sooner.
            nc.sync.dma_start(wt[:, :, :, 0:128], w_d[:, :, :, 0:128])
            epsb = wp.tile([128, 1], F32, tag="eps")
            nc.vector.memset(epsb[:], 4.0 * EPS)
            # PE warm-up: dummy matmuls on a zeroed tile keep the PE busy
            # through the HAM activity window (~3.4us) while the first input
            # DMAs land, so real matmuls start at full clock.
            zt = wp.tile([128, 128], BF16, tag="warm_src")
            nc.vector.memset(zt[:], 0.0)
            warm_ps = pp.tile([128, NCOL], F32, tag="ps", name="warm_ps")
            for _ in range(30):
                nc.tensor.matmul(warm_ps[:, 0:128], zt[:], zt[:])

            for rep in range(reps):
              for n in range(N_PER_CORE):
                  # three row-bands (rows 42b .. 42b+44), double-buffered so
                  # the next image's bands prefetch during this image's tail.
                  # Band 0 is split 5/18/22 so the first row-group's rows
                  # land fast and matmuls start ~1 us in (subtile deps).
                  bands = []
                  for b in range(3):
                      if rep == 0 and n == 0 and b == 1:
                          # deferred second weight half (see above)
                          nc.sync.dma_start(
                              wt[:, :, :, 128:256], w_d[:, :, :, 128:256]
                          )
                      bt = xp.tile([C_IN, 45, W], BF16, tag="x", name=f"xb{b}")
                      rows = 45 if b < 2 else 44
                      if b == 0:
                          nc.sync.dma_start(bt[:, 0:5, :], x_d[n, :, 0:5, :])
                          nc.sync.dma_start(bt[:, 5:23, :], x_d[n, :, 5:23, :])
                      else:
                          nc.sync.dma_start(
                              bt[:, 0:23, :], x_d[n, :, 42 * b : 42 * b + 23, :]
                          )
                      nc.sync.dma_start(
                          bt[:, 23:rows, :], x_d[n, :, 42 * b + 23 : 42 * b + rows, :]
                      )
                      bands.append(bt)

                  for c in range(2):
                      last_chunk = rep == reps - 1 and n == N_PER_CORE - 1 and c == 1
                      sums = accp.tile([128, N_RG], F32, tag="acc")
                      sqs = accp.tile([128, N_RG], F32, tag="acc")
                      yblocks = []
                      for bg in range(N_BG):
                          yb = yp.tile([128, BG, NCOL], BF16, tag="y")
                          # tap-inner ordering: each PSUM tile finishes 9 MMs
                          # after the previous, so evacuation streams
                          # alongside the PE (bf16 FWL hides the per-MM
                          # weight reload behind the streaming matmul)
                          for j in range(BG):
                              g = bg * BG + j
                              band = bands[bg // 2]
                              ps = pp.tile([128, NCOL], F32, tag="ps", name=f"ps{bg}_{j}")
                              for t in range(KH * KW):
                                  kh, kw = divmod(t, KW)
                                  lhs = wt[:, kh, kw, c * 128 : (c + 1) * 128]
                                  r0 = RG * g + kh - 42 * (bg // 2)
                                  rhs = band[:, r0 : r0 + RG, kw : kw + WO]
                                  nc.tensor.matmul(
                                      ps[:],
                                      lhs,
                                      rhs,
                                      start=(t == 0),
                                      stop=(t == KH * KW - 1),
                                  )
                              # ACT evacuates + accumulates sum(y); DVE
                              # computes sum(y^2) in one fused pass off the
                              # SBUF copy, freeing the PSUM bank after a
                              # single ACT read.
                              nc.scalar.activation(
                                  yb[:, j, :],
                                  ps[:],
                                  Copy,
                                  accum_out=sums[:, g : g + 1],
                              )
                              sq = sqp.tile([128, NCOL], BF16, tag="sq", name=f"sq{j}")
                              nc.vector.scalar_tensor_tensor(
                                  sq[:],
                                  yb[:, j, :],
                                  1.0,
                                  yb[:, j, :],
                                  op0=mult,
                                  op1=mult,
                                  accum_out=sqs[:, g : g + 1],
                              )
                          yblocks.append(yb)

                      # Per-(n, channel) stats over all 15876 pixels.
                      # Chain kept short: it is on the critical path for the
                      # final chunk's (pure-tail) normalize + store.
                      s1 = stp.tile([128, 1], F32, tag="st")
                      nc.vector.tensor_reduce(
                          s1[:], sums[:], axis=mybir.AxisListType.X, op=add
                      )
                      s2 = stp.tile([128, 1], F32, tag="st")
                      nc.vector.tensor_reduce(
                          s2[:], sqs[:], axis=mybir.AxisListType.X, op=add
                      )
                      mean = stp.tile([128, 1], F32, tag="st")
                      nc.vector.tensor_scalar_mul(mean[:], s1[:], 1.0 / PIX)
                      msq = stp.tile([128, 1], F32, tag="st")
                      nc.vector.tensor_mul(msq[:], mean[:], mean[:])
                      var = stp.tile([128, 1], F32, tag="st")
                      nc.vector.scalar_tensor_tensor(
                          var[:], s2[:], 1.0 / PIX, msq[:],
                          op0=mult, op1=subtract,
                      )
                      # alpha = rsqrt(var+eps)/2 = 1/sqrt(4*var + 4*eps)
                      std2 = stp.tile([128, 1], F32, tag="st")
                      nc.scalar.activation(
                          std2[:], var[:], Sqrt, bias=epsb[:], scale=4.0
                      )
                      alpha = stp.tile([128, 1], F32, tag="st")
                      nc.vector.reciprocal(alpha[:], std2[:])
                      nmalpha = stp.tile([128, 1], F32, tag="st")
                      nc.vector.scalar_tensor_tensor(
                          nmalpha[:], mean[:], -1.0, alpha[:],
                          op0=mult, op1=mult,
                      )

                      for bg, yb in enumerate(yblocks):
                          for j in range(BG):
                              if last_chunk and j % 3 == 2:
                                  nc.scalar.activation(
                                      yb[:, j, :],
                                      yb[:, j, :],
                                      Identity,
                                      bias=nmalpha[:],
                                      scale=alpha[:],
                                  )
                              else:
                                  nc.vector.tensor_scalar(
                                      yb[:, j, :],
                                      yb[:, j, :],
                                      alpha[:],
                                      nmalpha[:],
                                      op0=mult,
                                      op1=add,
                                  )
                              if last_chunk and j == 3:
                                  # early half-group store: the tail DMA is
                                  # bandwidth-bound, start it sooner
                                  nc.sync.dma_start(
                                      o_d[
                                          n,
                                          c * 128 : (c + 1) * 128,
                                          bg * BG * RG : bg * BG * RG + 4 * RG,
                                          :,
                                      ],
                                      yb[:, 0:4, :],
                                  )
                          if last_chunk:
                              nc.sync.dma_start(
                                  o_d[
                                      n,
                                      c * 128 : (c + 1) * 128,
                                      bg * BG * RG + 4 * RG : (bg + 1) * BG * RG,
                                      :,
                                  ],
                                  yb[:, 4:BG, :],
                              )
                          else:
                              nc.sync.dma_start(
                                  o_d[
                                      n,
                                      c * 128 : (c + 1) * 128,
                                      bg * BG * RG : (bg + 1) * BG * RG,
                                      :,
                                  ],
                                  yb[:],
                              )
    return nc


def _per_core_inputs(x, weight):
    """Shard + dtype-convert FULL inputs into the per-core in_map lists."""
    x16 = np.ascontiguousarray(np.asarray(x)).astype(NP_BF16)
    wt = np.ascontiguousarray(
        np.asarray(weight, dtype=np.float32).transpose(1, 2, 3, 0)
    ).astype(NP_BF16)
    return {
        "x": [x16[c * N_PER_CORE : (c + 1) * N_PER_CORE] for c in range(N_CORES)],
        "w": [wt] * N_CORES,
    }


_CACHED = None


def _get_exec(reps=1):
    """Build the Bass program once and wrap it in a persistent jitted
    shard_map executor (mirrors bass2jax.run_bass_via_pjrt, but without
    donation so the callable can be re-invoked for timing)."""
    global _CACHED
    if _CACHED is not None and _CACHED[5] == reps:
        return _CACHED

    import jax
    from jax.experimental.shard_map import shard_map
    from jax.sharding import Mesh, PartitionSpec

    from concourse import bass2jax

    bass2jax.install_neuronx_cc_hook()
    nc = _build_nc(reps)

    partition_name = (
        nc.partition_id_tensor.name if nc.partition_id_tensor else None
    )
    in_names = []
    out_names = []
    out_avals = []
    for alloc in nc.m.functions[0].allocations:
        if not isinstance(alloc, mybir.MemoryLocationSet):
            continue
        name = alloc.memorylocations[0].name
        if alloc.kind == "ExternalInput":
            if name != partition_name:
                in_names.append(name)
        elif alloc.kind == "ExternalOutput":
            out_names.append(name)
            out_avals.append(
                jax.core.ShapedArray(
                    tuple(alloc.tensor_shape), mybir.dt.np(alloc.dtype)
                )
            )
    n_params = len(in_names)
    all_in_names = in_names + out_names
    if partition_name is not None:
        all_in_names = all_in_names + [partition_name]

    def _body(*args):
        operands = list(args)
        if partition_name is not None:
            operands.append(bass2jax.partition_id_tensor())
        outs = bass2jax._bass_exec_p.bind(
            *operands,
            out_avals=tuple(out_avals),
            in_names=tuple(all_in_names),
            out_names=tuple(out_names),
            lowering_input_output_aliases=(),
            sim_require_finite=True,
            sim_require_nnan=True,
            nc=nc,
        )
        return tuple(outs)

    devices = jax.devices()[:N_CORES]
    mesh = Mesh(np.asarray(devices), ("core",))
    n_outs = len(out_names)
    sharded = jax.jit(
        shard_map(
            _body,
            mesh=mesh,
            in_specs=(PartitionSpec("core"),) * (n_params + n_outs),
            out_specs=(PartitionSpec("core"),) * n_outs,
            check_rep=False,
        ),
        keep_unused=True,
    )
    zeros = [
        np.zeros((N_CORES * a.shape[0], *a.shape[1:]), a.dtype) for a in out_avals
    ]
    _CACHED = (sharded, in_names, out_names, out_avals, zeros, reps)
    return _CACHED


def _run(per_core_inputs):
    """per_core_inputs: dict name -> list of 8 per-core arrays.
    Returns dict name -> list of 8 per-core outputs."""
    sharded, in_names, out_names, out_avals, zeros, _ = _get_exec()
    concat_in = [
        np.concatenate([np.asarray(per_core_inputs[nm][c]) for c in range(N_CORES)], axis=0)
        for nm in in_names
    ]
    out_arrs = sharded(*concat_in, *zeros)
    return {
        nm: np.asarray(out_arrs[i]).reshape(N_CORES, *out_avals[i].shape)
        for i, nm in enumerate(out_names)
    }


def kernel(x, weight, bias):
    # bias is mathematically a no-op under InstanceNorm(affine=False).
    del bias
    per_core = _per_core_inputs(x, weight)
    outs = _run(per_core)["out"]  # [8, 2, 256, 126, 126] bf16
    return outs.reshape(N, C_OUT, HO, WO).astype(np.float32)
